# revision 18
# baseline (speedup 1.0000x reference)
"""Binary-weight 3x3 conv (BinaryConv2d) Trainium2 Bass kernel.

Reference computation (for x[32,256,56,56] f32, w[256,256,3,3] f32, b[256] f32):
    out = conv2d(x, sign(w), pad=1) + sign(b)[None,:,None,None]

Strategy:
  - Data-parallel over batch: 8 cores x 4 images each. No collectives.
  - Conv expressed as 9 shifted matmuls (taps) over a zero-padded SBUF image,
    contracting input channels (2 chunks of 128) into PSUM.
  - x is split into bf16 hi + lo (lo = x - hi, exact by Sterbenz); both passes
    accumulate -> fp32-grade accuracy at bf16 PE throughput (weights are
    exactly +-1 in bf16/fp8).
  - mode "fp8lo": the lo pass runs in fp8e4m3 with perf_mode=DoubleRow, which
    contracts both 128-channel chunks in one matmul (9 MMs instead of 18 for
    the lo pass). lo is pre-scaled by 512 so values sit in e4m3's normal
    range; the combine step scales back by 1/512.
  - Weights are binarized on-chip (ACT Sign) and transposed per-tap via the PE
    transpose path into [in_ch, out_ch] stationary tiles.
  - Output: PSUM -> SBUF with per-partition binarized bias, then DMA out.
"""

from contextlib import ExitStack

import numpy as np

import concourse.bacc as bacc
import concourse.bass as bass
import concourse.tile as tile
import concourse.mybir as mybir
from concourse import masks
from concourse.bass_utils import run_bass_kernel_spmd

F32 = mybir.dt.float32
BF16 = mybir.dt.bfloat16
FP8 = mybir.dt.float8e4

N_CORES = 8
B, C, H, W = 32, 256, 56, 56
O = 256
KH = KW = 3
BPC = B // N_CORES  # images per core

ROWS_PER_TILE = 8          # output rows per PSUM tile (8*56 = 448 <= 512 free)
KI = C // 128              # input-channel chunks (contraction)
OC = O // 128              # output-channel chunks
LO_SCALE = 512.0           # fp8 lo pre-scale (2^9, exact in fp)

# "f32r": single-pass fp32r matmuls — 1 PE cycle/row at N>=256 (same rate as
# bf16) with ~19-bit effective input mantissa on TRN2 silicon. Measured on HW:
# 213 us/iter, scale-relative absmax 1.04e-4 vs the fp32 reference.
# "bf16": hi/lo two-pass — 397 us/iter, 2.7e-6 (use if a tighter-than-1e-4
# accuracy gate is ever required).
# "fp8hl": hi/lo two-pass, both in fp8e4m3 DoubleRow (contract 256 ch per MM
# at 0.5 PE cycles/output-row). hi = e4m3(x), lo = e4m3(x - hi); both passes
# use the same +-1 stationary weights and accumulate into one PSUM chain, so
# the combine is a single biased ACT drain. Simulated rel err 7.8e-4.
MODE = "fp8hl"             # "bf16" | "fp8lo" | "f32r" | "fp8hl"


def _build_fp8hl(bpc=BPC, h=H, w=W, repeat=1):
    """hi/lo fp8e4m3 DoubleRow build: 18 MMs per output tile, one PSUM chain.

    hi = e4m3(x), lo = e4m3(x - hi). Both passes use the same +-1 stationary
    weights; DoubleRow contracts both 128-channel chunks per MM. PSUM ends up
    holding conv(hi) + conv(lo) = conv(x) to ~8-bit-mantissa accuracy, drained
    once through ACT with the binarized bias.
    """
    fw = w + 1                       # wrap pitch: col w doubles as L/R pad
    flat = (h + 2) * fw + 2
    flat_pad = -flat % 16
    n_row_chunks = max(1, h // ROWS_PER_TILE)
    rows = h // n_row_chunks

    nc = bacc.Bacc("TRN2", target_bir_lowering=False, debug=False,
                   num_devices=N_CORES)
    x_d = nc.dram_tensor("x", [bpc, C, h, w], F32, kind="ExternalInput").ap()
    w_d = nc.dram_tensor("weight", [O, C, KH, KW], F32,
                         kind="ExternalInput").ap()
    b_d = nc.dram_tensor("bias", [O], F32, kind="ExternalInput").ap()
    o_d = nc.dram_tensor("out", [bpc, O, h, w], F32, kind="ExternalOutput").ap()

    with tile.TileContext(nc) as tc, ExitStack() as ctx:
        const = ctx.enter_context(tc.tile_pool(name="const", bufs=1))
        wstg_p = ctx.enter_context(tc.tile_pool(name="wstg", bufs=2))
        xstg_p = ctx.enter_context(tc.tile_pool(name="xstg", bufs=4))
        xpad_p = ctx.enter_context(tc.tile_pool(name="xpad", bufs=2))
        out_p = ctx.enter_context(tc.tile_pool(name="outp", bufs=6))

        identity = const.tile([128, 128], BF16)
        masks.make_identity(nc, identity[:])

        bias_raw = const.tile([128, OC], F32)
        bias_bin = const.tile([128, OC], F32)
        nc.sync.dma_start(out=bias_raw[:],
                          in_=b_d.rearrange("(b a) -> a b", b=OC))
        nc.scalar.sign(bias_bin[:], bias_raw[:])

        # lhsT8[:, (ky*KW+kx)*OC+oc, ki, :] = sign(W[oc, ki, ky, kx]).T in fp8
        lhsT8 = const.tile([128, KH * KW * OC, KI, 128], FP8)

        tpsum_ctx = ExitStack()
        tpsum_p = tpsum_ctx.enter_context(
            tc.tile_pool(name="tpsum", bufs=2, space=bass.MemorySpace.PSUM))
        for oc in range(OC):
            # one DMA per oc-chunk: DRAM runs are 256*9 floats per out
            # channel (9216 B descriptors; per-(oc,ki) loads would be 36 B)
            wstg = wstg_p.tile([128, C, KH, KW], F32, tag="wstg")
            nc.sync.dma_start(
                out=wstg[:], in_=w_d[oc * 128:(oc + 1) * 128, :, :, :])
            wbin = wstg_p.tile([128, C, KH, KW], BF16, tag="wbin")
            nc.scalar.sign(wbin[:], wstg[:])
            for ki in range(KI):
                for ky in range(KH):
                    for kx in range(KW):
                        tp = tpsum_p.tile([128, 128], BF16)
                        nc.tensor.transpose(
                            tp[:], wbin[:, ki * 128:(ki + 1) * 128, ky, kx],
                            identity[:])
                        j = (ky * KW + kx) * OC + oc
                        nc.vector.tensor_copy(lhsT8[:, j, ki, :], tp[:])
        tpsum_ctx.close()

        psum_p = ctx.enter_context(
            tc.tile_pool(name="psum", bufs=7, space=bass.MemorySpace.PSUM))

        for _rep in range(repeat):
            for n in range(bpc):
                hi8 = xpad_p.tile([128, KI, flat + flat_pad], FP8, tag="hi8")
                lo8 = xpad_p.tile([128, KI, flat + flat_pad], FP8, tag="lo8")
                for t in (hi8, lo8):
                    nc.gpsimd.memset(t[:, :, 0:fw + 1], 0.0)
                    nc.gpsimd.memset(t[:, :, (h + 1) * fw + 1:], 0.0)
                for ki in range(KI):
                    xf = xstg_p.tile([128, h, w], F32, tag="xf")
                    nc.sync.dma_start(
                        out=xf[:],
                        in_=x_d[n, ki * 128:(ki + 1) * 128, :, :])
                    hib = hi8[:, ki, fw + 1:(h + 1) * fw + 1].rearrange(
                        "p (r c) -> p r c", c=fw)
                    lob = lo8[:, ki, fw + 1:(h + 1) * fw + 1].rearrange(
                        "p (r c) -> p r c", c=fw)
                    nc.gpsimd.memset(hib[:, :, w:fw], 0.0)
                    nc.gpsimd.memset(lob[:, :, w:fw], 0.0)
                    nc.scalar.copy(hib[:, :, 0:w], xf[:])
                    # lo = (x * 1.0) - hi, converted to fp8 on the way out
                    nc.vector.scalar_tensor_tensor(
                        lob[:, :, 0:w], xf[:], 1.0, hib[:, :, 0:w],
                        op0=mybir.AluOpType.mult,
                        op1=mybir.AluOpType.subtract)

                for oc in range(OC):
                    # tap-outer order: all 7 row-chunks x {hi,lo} stream
                    # against ONE stationary before it changes (14 MMs per
                    # weight load instead of 1) — amortizes the PE stationary
                    # load, which does not fully hide behind 228-cycle
                    # streams. Each row-chunk accumulates in its own PSUM
                    # bank across the 18-MM group.
                    pss = [psum_p.tile([128, rows * fw], F32,
                                       name="ps", tag="ps")
                           for rc in range(n_row_chunks)]
                    for t in range(KH * KW):
                        ky, kx = divmod(t, KW)
                        j = t * OC + oc
                        for src_i, src in ((0, hi8), (1, lo8)):
                            for rc in range(n_row_chunks):
                                s = (rc * rows + ky) * fw + kx
                                nc.tensor.matmul(
                                    pss[rc][:], lhsT8[:, j, :, :],
                                    src[:, :, s:s + rows * fw],
                                    start=(t == 0 and src_i == 0),
                                    stop=(t == KH * KW - 1 and src_i == 1),
                                    perf_mode=mybir.MatmulPerfMode.DoubleRow)
                    for rc in range(n_row_chunks):
                        r0 = rc * rows
                        ob = out_p.tile([128, rows, w], F32, tag="ob")
                        psv = pss[rc][:].rearrange("p (r c) -> p r c", c=fw)
                        nc.scalar.activation(
                            ob[:], psv[:, :, 0:w],
                            mybir.ActivationFunctionType.Identity,
                            bias=bias_bin[:, oc:oc + 1], scale=1.0)
                        nc.sync.dma_start(
                            out=o_d[n, oc * 128:(oc + 1) * 128,
                                    r0:r0 + rows, :],
                            in_=ob[:])

    nc.compile()
    return nc


def build_program(bpc=BPC, h=H, w=W, repeat=1, mode=None):
    """Build the per-core Bass program. Returns compiled nc."""
    mode = MODE if mode is None else mode
    if mode == "fp8hl":
        return _build_fp8hl(bpc=bpc, h=h, w=w, repeat=repeat)
    ph, pw = h + 2, w + 4
    n_row_chunks = max(1, h // ROWS_PER_TILE)
    rows = h // n_row_chunks

    nc = bacc.Bacc("TRN2", target_bir_lowering=False, debug=False,
                   num_devices=N_CORES)
    x_d = nc.dram_tensor("x", [bpc, C, h, w], F32, kind="ExternalInput").ap()
    w_d = nc.dram_tensor("weight", [O, C, KH, KW], F32,
                         kind="ExternalInput").ap()
    b_d = nc.dram_tensor("bias", [O], F32, kind="ExternalInput").ap()
    o_d = nc.dram_tensor("out", [bpc, O, h, w], F32, kind="ExternalOutput").ap()

    with tile.TileContext(nc) as tc, ExitStack() as ctx:
        const = ctx.enter_context(tc.tile_pool(name="const", bufs=1))
        wstg_p = ctx.enter_context(tc.tile_pool(name="wstg", bufs=2))
        xstg_p = ctx.enter_context(tc.tile_pool(name="xstg", bufs=5))
        hif_p = ctx.enter_context(tc.tile_pool(name="hif", bufs=2))
        xpad_p = ctx.enter_context(tc.tile_pool(name="xpad", bufs=2))
        out_p = ctx.enter_context(tc.tile_pool(name="outp", bufs=4))
        npsA = 6 if mode == "bf16" else 3

        # ---- constants ----
        # f32r mode: both matmul operands must be f32r (walrus rejects mixed
        # 32-bit/non-32-bit); weights are engine-rounded to f32r (+-1 exact).
        F32R = mybir.dt.float32r
        wdt = F32 if mode == "f32r" else BF16
        ldt = F32R if mode == "f32r" else BF16
        identity = const.tile([128, 128], wdt)
        masks.make_identity(nc, identity[:])

        bias_raw = const.tile([128, OC], F32)
        bias_bin = const.tile([128, OC], F32)
        # bias_raw[p, oc] = bias[oc*128 + p]
        nc.sync.dma_start(out=bias_raw[:],
                          in_=b_d.rearrange("(b a) -> a b", b=OC))
        nc.scalar.sign(bias_bin[:], bias_raw[:])

        # ---- weights: load, binarize, transpose per tap ----
        # lhsT_all[:, idx, :] = sign(W[oc_chunk, ki_chunk, tap]).T  (shape [i,o])
        lhsT_all = const.tile([128, KI * KH * KW * OC, 128], ldt)

        def lidx(ki, ky, kx, oc):
            return ((ki * KH + ky) * KW + kx) * OC + oc

        tpsum_ctx = ExitStack()
        tpsum_p = tpsum_ctx.enter_context(
            tc.tile_pool(name="tpsum", bufs=2, space=bass.MemorySpace.PSUM))
        for ki in range(KI):
            for oc in range(OC):
                wstg = wstg_p.tile([128, 128, KH, KW], F32, tag="wstg")
                nc.sync.dma_start(
                    out=wstg[:],
                    in_=w_d[oc * 128:(oc + 1) * 128, ki * 128:(ki + 1) * 128, :, :])
                wbin = wstg_p.tile([128, 128, KH, KW], wdt, tag="wbin")
                nc.scalar.sign(wbin[:], wstg[:])
                for ky in range(KH):
                    for kx in range(KW):
                        tp = tpsum_p.tile([128, 128], wdt)
                        nc.tensor.transpose(tp[:], wbin[:, :, ky, kx], identity[:])
                        nc.vector.tensor_copy(
                            lhsT_all[:, lidx(ki, ky, kx, oc), :], tp[:])

        if mode == "fp8lo":
            # lhsT8[:, j, ki, :] with j = (ky*KW+kx)*OC+oc : fp8 copies of the
            # per-tap transposed weights, ki-chunks adjacent for DoubleRow.
            lhsT8 = const.tile([128, KH * KW * OC, KI, 128], FP8)
            for ki in range(KI):
                for oc in range(OC):
                    for ky in range(KH):
                        for kx in range(KW):
                            j = (ky * KW + kx) * OC + oc
                            nc.vector.tensor_copy(
                                lhsT8[:, j, ki, :],
                                lhsT_all[:, lidx(ki, ky, kx, oc), :])

        tpsum_ctx.close()
        psum_p = ctx.enter_context(
            tc.tile_pool(name="psum", bufs=npsA, space=bass.MemorySpace.PSUM))
        if mode == "fp8lo":
            psumB_p = ctx.enter_context(
                tc.tile_pool(name="psumB", bufs=3, space=bass.MemorySpace.PSUM))

        # ---- main loop over images ----
        for _rep in range(repeat):
            for n in range(bpc):
                xpad = {}
                lo8 = None
                if mode == "fp8lo":
                    # Flat 57-pitch wrap layout per chunk: buffer index of
                    # x[r, c] is 1 + (r+1)*57 + c; the zero column at c=56 of
                    # each row doubles as right pad of row r and (via wrap)
                    # left pad of row r+1. Leading/trailing 57-blocks are the
                    # vertical zero rows. DoubleRow rhs slices are 3-D
                    # [128, KI, 8*57] contiguous per chunk.
                    fw = w + 1
                    flat = (h + 2) * fw + 2
                    flat_pad = -flat % 16
                    lo8 = xpad_p.tile([128, KI, flat + flat_pad], FP8,
                                      tag="lo8")
                    nc.gpsimd.memset(lo8[:, :, 0:fw + 1], 0.0)
                    nc.gpsimd.memset(lo8[:, :, (h + 1) * fw + 1:], 0.0)
                    for ki in range(KI):
                        body = lo8[:, ki, fw + 1:(h + 1) * fw + 1].rearrange(
                            "p (r c) -> p r c", c=fw)
                        nc.gpsimd.memset(body[:, :, w:fw], 0.0)
                if mode == "f32r":
                    # Single-pass fp32r: x is rounded to f32r by an ACT copy
                    # into the padded tile (the BIR verifier requires f32r
                    # matmul inputs to be engine-rounded, not raw DMA).
                    for ki in range(KI):
                        xf = xstg_p.tile([128, h, w], F32, tag="xf")
                        # two half-loads -> two DMA queues in parallel
                        hh = h // 2
                        nc.sync.dma_start(
                            out=xf[:, :hh, :],
                            in_=x_d[n, ki * 128:(ki + 1) * 128, :hh, :])
                        nc.sync.dma_start(
                            out=xf[:, hh:, :],
                            in_=x_d[n, ki * 128:(ki + 1) * 128, hh:, :])
                        xp = xpad_p.tile([128, ph, pw], F32R, tag=f"x{ki}")
                        xpf = xp[:].bitcast(F32)
                        nc.gpsimd.memset(xpf[:, 0, :], 0.0)
                        nc.gpsimd.memset(xpf[:, ph - 1, :], 0.0)
                        nc.gpsimd.memset(xpf[:, 1:ph - 1, 0], 0.0)
                        nc.gpsimd.memset(xpf[:, 1:ph - 1, w + 1:pw], 0.0)
                        for rc in range(n_row_chunks):
                            a, b = rc * rows, rc * rows + rows
                            nc.scalar.copy(xp[:, 1 + a:1 + b, 1:w + 1],
                                           xf[:, a:b, :])
                        xpad[("hi", ki)] = xp
                    for rc in range(n_row_chunks):
                        for oc in range(OC):
                            r0 = rc * rows
                            ps = psum_p.tile([128, rows, w], F32)
                            k = 0
                            nmm = KI * KH * KW
                            for ki in range(KI):
                                xp = xpad[("hi", ki)]
                                for ky in range(KH):
                                    for kx in range(KW):
                                        nc.tensor.matmul(
                                            ps[:],
                                            lhsT_all[:, lidx(ki, ky, kx, oc), :],
                                            xp[:, r0 + ky:r0 + ky + rows,
                                               kx:kx + w],
                                            start=(k == 0),
                                            stop=(k == nmm - 1))
                                        k += 1
                            ob = out_p.tile([128, rows, w], F32)
                            nc.scalar.activation(
                                ob[:], ps[:],
                                mybir.ActivationFunctionType.Identity,
                                bias=bias_bin[:, oc:oc + 1], scale=1.0)
                            nc.sync.dma_start(
                                out=o_d[n, oc * 128:(oc + 1) * 128,
                                        r0:r0 + rows, :],
                                in_=ob[:])
                    continue
                for ki in range(KI):
                    xf = xstg_p.tile([128, h, w], F32, tag="xf")
                    nc.sync.dma_start(out=xf[:],
                                      in_=x_d[n, ki * 128:(ki + 1) * 128, :, :])
                    hi = xpad_p.tile([128, ph, pw], BF16, tag=f"hi{ki}")
                    nc.gpsimd.memset(hi[:, 0, :], 0.0)
                    nc.gpsimd.memset(hi[:, ph - 1, :], 0.0)
                    nc.gpsimd.memset(hi[:, 1:ph - 1, 0], 0.0)
                    nc.gpsimd.memset(hi[:, 1:ph - 1, w + 1:pw], 0.0)
                    xpad[("hi", ki)] = hi
                    if mode == "bf16":
                        lo = xpad_p.tile([128, ph, pw], BF16, tag=f"lo{ki}")
                        nc.gpsimd.memset(lo[:, 0, :], 0.0)
                        nc.gpsimd.memset(lo[:, ph - 1, :], 0.0)
                        nc.gpsimd.memset(lo[:, 1:ph - 1, 0], 0.0)
                        nc.gpsimd.memset(lo[:, 1:ph - 1, w + 1:pw], 0.0)
                        # Chunked by row group so downstream matmuls can start
                        # before the whole image is converted, and so PSUM
                        # drains never queue behind a multi-us engine op.
                        for rc in range(n_row_chunks):
                            a, b = rc * rows, rc * rows + rows
                            # hi = bf16(x)
                            nc.scalar.copy(hi[:, 1 + a:1 + b, 1:w + 1],
                                           xf[:, a:b, :])
                            # lo = bf16(x - hi)   (x - hi exact by Sterbenz)
                            nc.vector.tensor_sub(lo[:, 1 + a:1 + b, 1:w + 1],
                                                 xf[:, a:b, :],
                                                 hi[:, 1 + a:1 + b, 1:w + 1])
                        xpad[("lo", ki)] = lo
                    else:
                        nc.scalar.copy(hi[:, 1:h + 1, 1:w + 1], xf[:])
                        hif = hif_p.tile([128, h, w], F32, tag="hif")
                        nc.scalar.copy(hif[:], hi[:, 1:h + 1, 1:w + 1])
                        tmp = hif_p.tile([128, h, w], F32, tag="tmp")
                        nc.vector.tensor_sub(tmp[:], xf[:], hif[:])
                        fw = w + 1
                        body = lo8[:, ki, fw + 1:(h + 1) * fw + 1].rearrange(
                            "p (r c) -> p r c", c=fw)
                        nc.vector.tensor_scalar_mul(
                            body[:, :, 0:w], tmp[:], LO_SCALE)

                for rc in range(n_row_chunks):
                    for oc in range(OC):
                        r0 = rc * rows
                        ps = psum_p.tile([128, rows, w], F32)
                        k = 0
                        if mode == "bf16":
                            nmm = 2 * KI * KH * KW
                            for p in ("hi", "lo"):
                                for ki in range(KI):
                                    xp = xpad[(p, ki)]
                                    for ky in range(KH):
                                        for kx in range(KW):
                                            nc.tensor.matmul(
                                                ps[:],
                                                lhsT_all[:, lidx(ki, ky, kx, oc), :],
                                                xp[:, r0 + ky:r0 + ky + rows,
                                                   kx:kx + w],
                                                start=(k == 0),
                                                stop=(k == nmm - 1))
                                            k += 1
                            ob = out_p.tile([128, rows, w], F32)
                            nc.scalar.activation(
                                ob[:], ps[:],
                                mybir.ActivationFunctionType.Identity,
                                bias=bias_bin[:, oc:oc + 1], scale=1.0)
                        else:
                            nmm = KI * KH * KW
                            for ki in range(KI):
                                xp = xpad[("hi", ki)]
                                for ky in range(KH):
                                    for kx in range(KW):
                                        nc.tensor.matmul(
                                            ps[:],
                                            lhsT_all[:, lidx(ki, ky, kx, oc), :],
                                            xp[:, r0 + ky:r0 + ky + rows,
                                               kx:kx + w],
                                            start=(k == 0),
                                            stop=(k == nmm - 1))
                                        k += 1
                            fw = w + 1
                            psB = psumB_p.tile([128, rows * fw], F32)
                            for j2, (ky, kx) in enumerate(
                                    (a, b) for a in range(KH) for b in range(KW)):
                                j = (ky * KW + kx) * OC + oc
                                s = (r0 + ky) * fw + kx
                                nc.tensor.matmul(
                                    psB[:],
                                    lhsT8[:, j, :, :],
                                    lo8[:, :, s:s + rows * fw],
                                    start=(j2 == 0),
                                    stop=(j2 == KH * KW - 1),
                                    perf_mode=mybir.MatmulPerfMode.DoubleRow)
                            # combine: out = hi_psum + lo_psum/512 + bias
                            tmp_sb = out_p.tile([128, rows, w], F32, tag="cmb")
                            psBv = psB[:].rearrange("p (r c) -> p r c", c=fw)
                            nc.scalar.activation(
                                tmp_sb[:], psBv[:, :, 0:w],
                                mybir.ActivationFunctionType.Identity,
                                bias=bias_bin[:, oc:oc + 1], scale=1.0 / LO_SCALE)
                            ob = out_p.tile([128, rows, w], F32)
                            nc.vector.tensor_add(ob[:], tmp_sb[:], ps[:])
                        nc.sync.dma_start(
                            out=o_d[n, oc * 128:(oc + 1) * 128, r0:r0 + rows, :],
                            in_=ob[:])

    nc.compile()
    return nc


_CACHE = {}


def _get_program():
    if "nc" not in _CACHE:
        _CACHE["nc"] = build_program()
    return _CACHE["nc"]


def kernel(x, weight, bias):
    x = np.ascontiguousarray(x, dtype=np.float32)
    weight = np.ascontiguousarray(weight, dtype=np.float32)
    bias = np.ascontiguousarray(bias, dtype=np.float32)
    nc = _get_program()
    in_maps = [
        {"x": x[c * BPC:(c + 1) * BPC], "weight": weight, "bias": bias}
        for c in range(N_CORES)
    ]
    r = run_bass_kernel_spmd(nc, in_maps, list(range(N_CORES)))
    return np.concatenate([r.results[c]["out"] for c in range(N_CORES)], axis=0)



# revision 20
# speedup vs baseline: 1.2832x; 1.2832x over previous
"""Binary-weight 3x3 conv (BinaryConv2d) Trainium2 Bass kernel.

Reference computation (for x[32,256,56,56] f32, w[256,256,3,3] f32, b[256] f32):
    out = conv2d(x, sign(w), pad=1) + sign(b)[None,:,None,None]

Strategy:
  - Data-parallel over batch: 8 cores x 4 images each. No collectives.
  - Conv expressed as 9 shifted matmuls (taps) over a zero-padded SBUF image,
    contracting input channels (2 chunks of 128) into PSUM.
  - x is split into bf16 hi + lo (lo = x - hi, exact by Sterbenz); both passes
    accumulate -> fp32-grade accuracy at bf16 PE throughput (weights are
    exactly +-1 in bf16/fp8).
  - mode "fp8lo": the lo pass runs in fp8e4m3 with perf_mode=DoubleRow, which
    contracts both 128-channel chunks in one matmul (9 MMs instead of 18 for
    the lo pass). lo is pre-scaled by 512 so values sit in e4m3's normal
    range; the combine step scales back by 1/512.
  - Weights are binarized on-chip (ACT Sign) and transposed per-tap via the PE
    transpose path into [in_ch, out_ch] stationary tiles.
  - Output: PSUM -> SBUF with per-partition binarized bias, then DMA out.
"""

from contextlib import ExitStack

import numpy as np

import concourse.bacc as bacc
import concourse.bass as bass
import concourse.tile as tile
import concourse.mybir as mybir
from concourse import masks
from concourse.bass_utils import run_bass_kernel_spmd

F32 = mybir.dt.float32
BF16 = mybir.dt.bfloat16
FP8 = mybir.dt.float8e4

N_CORES = 8
B, C, H, W = 32, 256, 56, 56
O = 256
KH = KW = 3
BPC = B // N_CORES  # images per core

ROWS_PER_TILE = 8          # output rows per PSUM tile (8*56 = 448 <= 512 free)
KI = C // 128              # input-channel chunks (contraction)
OC = O // 128              # output-channel chunks
LO_SCALE = 512.0           # fp8 lo pre-scale (2^9, exact in fp)

# "f32r": single-pass fp32r matmuls — 1 PE cycle/row at N>=256 (same rate as
# bf16) with ~19-bit effective input mantissa on TRN2 silicon. Measured on HW:
# 213 us/iter, scale-relative absmax 1.04e-4 vs the fp32 reference.
# "bf16": hi/lo two-pass — 397 us/iter, 2.7e-6 (use if a tighter-than-1e-4
# accuracy gate is ever required).
# "fp8hl": hi/lo two-pass, both in fp8e4m3 DoubleRow (contract 256 ch per MM
# at 0.5 PE cycles/output-row). hi = e4m3(x), lo = e4m3(x - hi); both passes
# use the same +-1 stationary weights and accumulate into one PSUM chain, so
# the combine is a single biased ACT drain. Simulated rel err 7.8e-4.
MODE = "fp8hl"             # "bf16" | "fp8lo" | "f32r" | "fp8hl"


def _build_fp8hl(bpc=BPC, h=H, w=W, repeat=1):
    """hi/lo fp8e4m3 DoubleRow build: 18 MMs per output tile, one PSUM chain.

    hi = e4m3(x), lo = e4m3(x - hi). Both passes use the same +-1 stationary
    weights; DoubleRow contracts both 128-channel chunks per MM. PSUM ends up
    holding conv(hi) + conv(lo) = conv(x) to ~8-bit-mantissa accuracy, drained
    once through ACT with the binarized bias.
    """
    fw = w + 1                       # wrap pitch: col w doubles as L/R pad
    flat = (h + 2) * fw + 2
    flat_pad = -flat % 16
    n_row_chunks = max(1, h // ROWS_PER_TILE)
    rows = h // n_row_chunks

    nc = bacc.Bacc("TRN2", target_bir_lowering=False, debug=False,
                   num_devices=N_CORES)
    x_d = nc.dram_tensor("x", [bpc, C, h, w], F32, kind="ExternalInput").ap()
    w_d = nc.dram_tensor("weight", [O, C, KH, KW], F32,
                         kind="ExternalInput").ap()
    b_d = nc.dram_tensor("bias", [O], F32, kind="ExternalInput").ap()
    o_d = nc.dram_tensor("out", [bpc, O, h, w], F32, kind="ExternalOutput").ap()

    with tile.TileContext(nc) as tc, ExitStack() as ctx:
        const = ctx.enter_context(tc.tile_pool(name="const", bufs=1))
        wstg_p = ctx.enter_context(tc.tile_pool(name="wstg", bufs=2))
        xstg_p = ctx.enter_context(tc.tile_pool(name="xstg", bufs=4))
        xpad_p = ctx.enter_context(tc.tile_pool(name="xpad", bufs=2))
        out_p = ctx.enter_context(tc.tile_pool(name="outp", bufs=6))

        identity = const.tile([128, 128], BF16)
        masks.make_identity(nc, identity[:])

        bias_raw = const.tile([128, OC], F32)
        bias_bin = const.tile([128, OC], F32)
        nc.sync.dma_start(out=bias_raw[:],
                          in_=b_d.rearrange("(b a) -> a b", b=OC))
        nc.scalar.sign(bias_bin[:], bias_raw[:])

        # lhsT8[:, (ky*KW+kx)*OC+oc, ki, :] = sign(W[oc, ki, ky, kx]).T in fp8
        lhsT8 = const.tile([128, KH * KW * OC, KI, 128], FP8)

        tpsum_ctx = ExitStack()
        tpsum_p = tpsum_ctx.enter_context(
            tc.tile_pool(name="tpsum", bufs=2, space=bass.MemorySpace.PSUM))
        for oc in range(OC):
            # one DMA per oc-chunk: DRAM runs are 256*9 floats per out
            # channel (9216 B descriptors; per-(oc,ki) loads would be 36 B)
            wstg = wstg_p.tile([128, C, KH, KW], F32, tag="wstg")
            nc.sync.dma_start(
                out=wstg[:], in_=w_d[oc * 128:(oc + 1) * 128, :, :, :])
            wbin = wstg_p.tile([128, C, KH, KW], BF16, tag="wbin")
            nc.scalar.sign(wbin[:], wstg[:])
            for ki in range(KI):
                for ky in range(KH):
                    for kx in range(KW):
                        tp = tpsum_p.tile([128, 128], BF16)
                        nc.tensor.transpose(
                            tp[:], wbin[:, ki * 128:(ki + 1) * 128, ky, kx],
                            identity[:])
                        j = (ky * KW + kx) * OC + oc
                        nc.vector.tensor_copy(lhsT8[:, j, ki, :], tp[:])
        tpsum_ctx.close()

        psum_p = ctx.enter_context(
            tc.tile_pool(name="psum", bufs=4, space=bass.MemorySpace.PSUM))

        for _rep in range(repeat):
            for n in range(bpc):
                hi8 = xpad_p.tile([128, KI, flat + flat_pad], FP8, tag="hi8")
                lo8 = xpad_p.tile([128, KI, flat + flat_pad], FP8, tag="lo8")
                for t in (hi8, lo8):
                    nc.gpsimd.memset(t[:, :, 0:fw + 1], 0.0)
                    nc.gpsimd.memset(t[:, :, (h + 1) * fw + 1:], 0.0)
                for ki in range(KI):
                    xf = xstg_p.tile([128, h, w], F32, tag="xf")
                    nc.sync.dma_start(
                        out=xf[:],
                        in_=x_d[n, ki * 128:(ki + 1) * 128, :, :])
                    hib = hi8[:, ki, fw + 1:(h + 1) * fw + 1].rearrange(
                        "p (r c) -> p r c", c=fw)
                    lob = lo8[:, ki, fw + 1:(h + 1) * fw + 1].rearrange(
                        "p (r c) -> p r c", c=fw)
                    nc.gpsimd.memset(hib[:, :, w:fw], 0.0)
                    nc.gpsimd.memset(lob[:, :, w:fw], 0.0)
                    nc.scalar.copy(hib[:, :, 0:w], xf[:])
                    # lo = (x * 1.0) - hi, converted to fp8 on the way out
                    nc.vector.scalar_tensor_tensor(
                        lob[:, :, 0:w], xf[:], 1.0, hib[:, :, 0:w],
                        op0=mybir.AluOpType.mult,
                        op1=mybir.AluOpType.subtract)

                for oc in range(OC):
                    for rc in range(n_row_chunks):
                        r0 = rc * rows
                        ps = psum_p.tile([128, rows * fw], F32)
                        k = 0
                        for ky in range(KH):
                            for kx in range(KW):
                                j = (ky * KW + kx) * OC + oc
                                s = (r0 + ky) * fw + kx
                                for src in (hi8, lo8):
                                    nc.tensor.matmul(
                                        ps[:], lhsT8[:, j, :, :],
                                        src[:, :, s:s + rows * fw],
                                        start=(k == 0),
                                        stop=(k == 2 * KH * KW - 1),
                                        perf_mode=mybir.MatmulPerfMode.DoubleRow)
                                    k += 1
                        ob = out_p.tile([128, rows, w], F32, tag="ob")
                        psv = ps[:].rearrange("p (r c) -> p r c", c=fw)
                        nc.scalar.activation(
                            ob[:], psv[:, :, 0:w],
                            mybir.ActivationFunctionType.Identity,
                            bias=bias_bin[:, oc:oc + 1], scale=1.0)
                        nc.sync.dma_start(
                            out=o_d[n, oc * 128:(oc + 1) * 128,
                                    r0:r0 + rows, :],
                            in_=ob[:])

    nc.compile()
    return nc


def build_program(bpc=BPC, h=H, w=W, repeat=1, mode=None):
    """Build the per-core Bass program. Returns compiled nc."""
    mode = MODE if mode is None else mode
    if mode == "fp8hl":
        return _build_fp8hl(bpc=bpc, h=h, w=w, repeat=repeat)
    ph, pw = h + 2, w + 4
    n_row_chunks = max(1, h // ROWS_PER_TILE)
    rows = h // n_row_chunks

    nc = bacc.Bacc("TRN2", target_bir_lowering=False, debug=False,
                   num_devices=N_CORES)
    x_d = nc.dram_tensor("x", [bpc, C, h, w], F32, kind="ExternalInput").ap()
    w_d = nc.dram_tensor("weight", [O, C, KH, KW], F32,
                         kind="ExternalInput").ap()
    b_d = nc.dram_tensor("bias", [O], F32, kind="ExternalInput").ap()
    o_d = nc.dram_tensor("out", [bpc, O, h, w], F32, kind="ExternalOutput").ap()

    with tile.TileContext(nc) as tc, ExitStack() as ctx:
        const = ctx.enter_context(tc.tile_pool(name="const", bufs=1))
        wstg_p = ctx.enter_context(tc.tile_pool(name="wstg", bufs=2))
        xstg_p = ctx.enter_context(tc.tile_pool(name="xstg", bufs=5))
        hif_p = ctx.enter_context(tc.tile_pool(name="hif", bufs=2))
        xpad_p = ctx.enter_context(tc.tile_pool(name="xpad", bufs=2))
        out_p = ctx.enter_context(tc.tile_pool(name="outp", bufs=4))
        npsA = 6 if mode == "bf16" else 3

        # ---- constants ----
        # f32r mode: both matmul operands must be f32r (walrus rejects mixed
        # 32-bit/non-32-bit); weights are engine-rounded to f32r (+-1 exact).
        F32R = mybir.dt.float32r
        wdt = F32 if mode == "f32r" else BF16
        ldt = F32R if mode == "f32r" else BF16
        identity = const.tile([128, 128], wdt)
        masks.make_identity(nc, identity[:])

        bias_raw = const.tile([128, OC], F32)
        bias_bin = const.tile([128, OC], F32)
        # bias_raw[p, oc] = bias[oc*128 + p]
        nc.sync.dma_start(out=bias_raw[:],
                          in_=b_d.rearrange("(b a) -> a b", b=OC))
        nc.scalar.sign(bias_bin[:], bias_raw[:])

        # ---- weights: load, binarize, transpose per tap ----
        # lhsT_all[:, idx, :] = sign(W[oc_chunk, ki_chunk, tap]).T  (shape [i,o])
        lhsT_all = const.tile([128, KI * KH * KW * OC, 128], ldt)

        def lidx(ki, ky, kx, oc):
            return ((ki * KH + ky) * KW + kx) * OC + oc

        tpsum_ctx = ExitStack()
        tpsum_p = tpsum_ctx.enter_context(
            tc.tile_pool(name="tpsum", bufs=2, space=bass.MemorySpace.PSUM))
        for ki in range(KI):
            for oc in range(OC):
                wstg = wstg_p.tile([128, 128, KH, KW], F32, tag="wstg")
                nc.sync.dma_start(
                    out=wstg[:],
                    in_=w_d[oc * 128:(oc + 1) * 128, ki * 128:(ki + 1) * 128, :, :])
                wbin = wstg_p.tile([128, 128, KH, KW], wdt, tag="wbin")
                nc.scalar.sign(wbin[:], wstg[:])
                for ky in range(KH):
                    for kx in range(KW):
                        tp = tpsum_p.tile([128, 128], wdt)
                        nc.tensor.transpose(tp[:], wbin[:, :, ky, kx], identity[:])
                        nc.vector.tensor_copy(
                            lhsT_all[:, lidx(ki, ky, kx, oc), :], tp[:])

        if mode == "fp8lo":
            # lhsT8[:, j, ki, :] with j = (ky*KW+kx)*OC+oc : fp8 copies of the
            # per-tap transposed weights, ki-chunks adjacent for DoubleRow.
            lhsT8 = const.tile([128, KH * KW * OC, KI, 128], FP8)
            for ki in range(KI):
                for oc in range(OC):
                    for ky in range(KH):
                        for kx in range(KW):
                            j = (ky * KW + kx) * OC + oc
                            nc.vector.tensor_copy(
                                lhsT8[:, j, ki, :],
                                lhsT_all[:, lidx(ki, ky, kx, oc), :])

        tpsum_ctx.close()
        psum_p = ctx.enter_context(
            tc.tile_pool(name="psum", bufs=npsA, space=bass.MemorySpace.PSUM))
        if mode == "fp8lo":
            psumB_p = ctx.enter_context(
                tc.tile_pool(name="psumB", bufs=3, space=bass.MemorySpace.PSUM))

        # ---- main loop over images ----
        for _rep in range(repeat):
            for n in range(bpc):
                xpad = {}
                lo8 = None
                if mode == "fp8lo":
                    # Flat 57-pitch wrap layout per chunk: buffer index of
                    # x[r, c] is 1 + (r+1)*57 + c; the zero column at c=56 of
                    # each row doubles as right pad of row r and (via wrap)
                    # left pad of row r+1. Leading/trailing 57-blocks are the
                    # vertical zero rows. DoubleRow rhs slices are 3-D
                    # [128, KI, 8*57] contiguous per chunk.
                    fw = w + 1
                    flat = (h + 2) * fw + 2
                    flat_pad = -flat % 16
                    lo8 = xpad_p.tile([128, KI, flat + flat_pad], FP8,
                                      tag="lo8")
                    nc.gpsimd.memset(lo8[:, :, 0:fw + 1], 0.0)
                    nc.gpsimd.memset(lo8[:, :, (h + 1) * fw + 1:], 0.0)
                    for ki in range(KI):
                        body = lo8[:, ki, fw + 1:(h + 1) * fw + 1].rearrange(
                            "p (r c) -> p r c", c=fw)
                        nc.gpsimd.memset(body[:, :, w:fw], 0.0)
                if mode == "f32r":
                    # Single-pass fp32r: x is rounded to f32r by an ACT copy
                    # into the padded tile (the BIR verifier requires f32r
                    # matmul inputs to be engine-rounded, not raw DMA).
                    for ki in range(KI):
                        xf = xstg_p.tile([128, h, w], F32, tag="xf")
                        # two half-loads -> two DMA queues in parallel
                        hh = h // 2
                        nc.sync.dma_start(
                            out=xf[:, :hh, :],
                            in_=x_d[n, ki * 128:(ki + 1) * 128, :hh, :])
                        nc.sync.dma_start(
                            out=xf[:, hh:, :],
                            in_=x_d[n, ki * 128:(ki + 1) * 128, hh:, :])
                        xp = xpad_p.tile([128, ph, pw], F32R, tag=f"x{ki}")
                        xpf = xp[:].bitcast(F32)
                        nc.gpsimd.memset(xpf[:, 0, :], 0.0)
                        nc.gpsimd.memset(xpf[:, ph - 1, :], 0.0)
                        nc.gpsimd.memset(xpf[:, 1:ph - 1, 0], 0.0)
                        nc.gpsimd.memset(xpf[:, 1:ph - 1, w + 1:pw], 0.0)
                        for rc in range(n_row_chunks):
                            a, b = rc * rows, rc * rows + rows
                            nc.scalar.copy(xp[:, 1 + a:1 + b, 1:w + 1],
                                           xf[:, a:b, :])
                        xpad[("hi", ki)] = xp
                    for rc in range(n_row_chunks):
                        for oc in range(OC):
                            r0 = rc * rows
                            ps = psum_p.tile([128, rows, w], F32)
                            k = 0
                            nmm = KI * KH * KW
                            for ki in range(KI):
                                xp = xpad[("hi", ki)]
                                for ky in range(KH):
                                    for kx in range(KW):
                                        nc.tensor.matmul(
                                            ps[:],
                                            lhsT_all[:, lidx(ki, ky, kx, oc), :],
                                            xp[:, r0 + ky:r0 + ky + rows,
                                               kx:kx + w],
                                            start=(k == 0),
                                            stop=(k == nmm - 1))
                                        k += 1
                            ob = out_p.tile([128, rows, w], F32)
                            nc.scalar.activation(
                                ob[:], ps[:],
                                mybir.ActivationFunctionType.Identity,
                                bias=bias_bin[:, oc:oc + 1], scale=1.0)
                            nc.sync.dma_start(
                                out=o_d[n, oc * 128:(oc + 1) * 128,
                                        r0:r0 + rows, :],
                                in_=ob[:])
                    continue
                for ki in range(KI):
                    xf = xstg_p.tile([128, h, w], F32, tag="xf")
                    nc.sync.dma_start(out=xf[:],
                                      in_=x_d[n, ki * 128:(ki + 1) * 128, :, :])
                    hi = xpad_p.tile([128, ph, pw], BF16, tag=f"hi{ki}")
                    nc.gpsimd.memset(hi[:, 0, :], 0.0)
                    nc.gpsimd.memset(hi[:, ph - 1, :], 0.0)
                    nc.gpsimd.memset(hi[:, 1:ph - 1, 0], 0.0)
                    nc.gpsimd.memset(hi[:, 1:ph - 1, w + 1:pw], 0.0)
                    xpad[("hi", ki)] = hi
                    if mode == "bf16":
                        lo = xpad_p.tile([128, ph, pw], BF16, tag=f"lo{ki}")
                        nc.gpsimd.memset(lo[:, 0, :], 0.0)
                        nc.gpsimd.memset(lo[:, ph - 1, :], 0.0)
                        nc.gpsimd.memset(lo[:, 1:ph - 1, 0], 0.0)
                        nc.gpsimd.memset(lo[:, 1:ph - 1, w + 1:pw], 0.0)
                        # Chunked by row group so downstream matmuls can start
                        # before the whole image is converted, and so PSUM
                        # drains never queue behind a multi-us engine op.
                        for rc in range(n_row_chunks):
                            a, b = rc * rows, rc * rows + rows
                            # hi = bf16(x)
                            nc.scalar.copy(hi[:, 1 + a:1 + b, 1:w + 1],
                                           xf[:, a:b, :])
                            # lo = bf16(x - hi)   (x - hi exact by Sterbenz)
                            nc.vector.tensor_sub(lo[:, 1 + a:1 + b, 1:w + 1],
                                                 xf[:, a:b, :],
                                                 hi[:, 1 + a:1 + b, 1:w + 1])
                        xpad[("lo", ki)] = lo
                    else:
                        nc.scalar.copy(hi[:, 1:h + 1, 1:w + 1], xf[:])
                        hif = hif_p.tile([128, h, w], F32, tag="hif")
                        nc.scalar.copy(hif[:], hi[:, 1:h + 1, 1:w + 1])
                        tmp = hif_p.tile([128, h, w], F32, tag="tmp")
                        nc.vector.tensor_sub(tmp[:], xf[:], hif[:])
                        fw = w + 1
                        body = lo8[:, ki, fw + 1:(h + 1) * fw + 1].rearrange(
                            "p (r c) -> p r c", c=fw)
                        nc.vector.tensor_scalar_mul(
                            body[:, :, 0:w], tmp[:], LO_SCALE)

                for rc in range(n_row_chunks):
                    for oc in range(OC):
                        r0 = rc * rows
                        ps = psum_p.tile([128, rows, w], F32)
                        k = 0
                        if mode == "bf16":
                            nmm = 2 * KI * KH * KW
                            for p in ("hi", "lo"):
                                for ki in range(KI):
                                    xp = xpad[(p, ki)]
                                    for ky in range(KH):
                                        for kx in range(KW):
                                            nc.tensor.matmul(
                                                ps[:],
                                                lhsT_all[:, lidx(ki, ky, kx, oc), :],
                                                xp[:, r0 + ky:r0 + ky + rows,
                                                   kx:kx + w],
                                                start=(k == 0),
                                                stop=(k == nmm - 1))
                                            k += 1
                            ob = out_p.tile([128, rows, w], F32)
                            nc.scalar.activation(
                                ob[:], ps[:],
                                mybir.ActivationFunctionType.Identity,
                                bias=bias_bin[:, oc:oc + 1], scale=1.0)
                        else:
                            nmm = KI * KH * KW
                            for ki in range(KI):
                                xp = xpad[("hi", ki)]
                                for ky in range(KH):
                                    for kx in range(KW):
                                        nc.tensor.matmul(
                                            ps[:],
                                            lhsT_all[:, lidx(ki, ky, kx, oc), :],
                                            xp[:, r0 + ky:r0 + ky + rows,
                                               kx:kx + w],
                                            start=(k == 0),
                                            stop=(k == nmm - 1))
                                        k += 1
                            fw = w + 1
                            psB = psumB_p.tile([128, rows * fw], F32)
                            for j2, (ky, kx) in enumerate(
                                    (a, b) for a in range(KH) for b in range(KW)):
                                j = (ky * KW + kx) * OC + oc
                                s = (r0 + ky) * fw + kx
                                nc.tensor.matmul(
                                    psB[:],
                                    lhsT8[:, j, :, :],
                                    lo8[:, :, s:s + rows * fw],
                                    start=(j2 == 0),
                                    stop=(j2 == KH * KW - 1),
                                    perf_mode=mybir.MatmulPerfMode.DoubleRow)
                            # combine: out = hi_psum + lo_psum/512 + bias
                            tmp_sb = out_p.tile([128, rows, w], F32, tag="cmb")
                            psBv = psB[:].rearrange("p (r c) -> p r c", c=fw)
                            nc.scalar.activation(
                                tmp_sb[:], psBv[:, :, 0:w],
                                mybir.ActivationFunctionType.Identity,
                                bias=bias_bin[:, oc:oc + 1], scale=1.0 / LO_SCALE)
                            ob = out_p.tile([128, rows, w], F32)
                            nc.vector.tensor_add(ob[:], tmp_sb[:], ps[:])
                        nc.sync.dma_start(
                            out=o_d[n, oc * 128:(oc + 1) * 128, r0:r0 + rows, :],
                            in_=ob[:])

    nc.compile()
    return nc


_CACHE = {}


def _get_program():
    if "nc" not in _CACHE:
        _CACHE["nc"] = build_program()
    return _CACHE["nc"]


def kernel(x, weight, bias):
    x = np.ascontiguousarray(x, dtype=np.float32)
    weight = np.ascontiguousarray(weight, dtype=np.float32)
    bias = np.ascontiguousarray(bias, dtype=np.float32)
    nc = _get_program()
    in_maps = [
        {"x": x[c * BPC:(c + 1) * BPC], "weight": weight, "bias": bias}
        for c in range(N_CORES)
    ]
    r = run_bass_kernel_spmd(nc, in_maps, list(range(N_CORES)))
    return np.concatenate([r.results[c]["out"] for c in range(N_CORES)], axis=0)

